# revision 1
# baseline (speedup 1.0000x reference)
"""Trainium2 Bass kernel for OldNeighborhoodEncoder (segment_reduce).

Math (reference):
    fc1    = relu(X @ W1.T + b1)            # [N, 64], X = [N, 3]
    pooled = segment_max(fc1, cluster, S)   # [S, 64], cluster = arange(N)//32
    h      = relu(pooled @ W1g.T + b1g)     # [S, 64]
    out    = relu(h @ W2g.T + b2g)          # [S, 128]

Hardcoded sizes: N=1048576, S=32768 (32 pts/cluster), FEATURE=64, FG0=64,
FG1=128, 8 cores. Data-parallel over points: core d handles points
[d*131072, (d+1)*131072) == clusters [d*4096, (d+1)*4096); no collectives.

Device layout (per core):
  xt [6, 65536]: col c = 512*g + o (g in 0..127, o in 0..511); rows 0-2 =
    xyz of point 1024*g + o, rows 3-5 = xyz of point 1024*g + 512 + o.
  wpack [6,128] = blockdiag(W1.T, W1.T): one matmul column-block computes
    fc1 (pre-bias) for TWO 512-point chunks at once -> full 128-partition
    PE output. Bias+relu are deferred past the max (monotone).
  psum [128,4,16,32]: bank b holds g = 4i+b; view [.., q, t] with o=32q+t,
    so a single DVE reduce over t pools 4*16 = 64 cluster-halves.
  pooled [128, 32, 4, 16]: pooled[64a+f, i, b, q] = max_z of cluster
    128i + 32b + 16a + q, feature f.
  Tail: relu(+b1) -> blockdiag(W1g.T) matmul -> relu(+b1g) ->
    W2g.T matmul (K=64, separately for a=0 from partitions 0:64 and a=1
    from 64:128) -> relu(+b2g) -> outA/outB [128, 2048].

v1.5 perf structure: the main loop is DVE-reduce-bound (Pool/GPSIMD has no
legal max op on this target, so DVE does all 32 chunk reductions); weight
DMAs go on the Scalar queue (HWDGE; gpsimd SWDGE blocked the first matmul
~7us); relu(+b1) of pooled happens in slices during the main loop on ACT;
the tail MLP is pipelined in 512-col sub-slices with relu work split
between ACT and DVE, and output DMAs are split in halves on two queues.
"""

import sys
import numpy as np

if "/opt/trn_rl_repo" not in sys.path:
    sys.path.insert(0, "/opt/trn_rl_repo")

N = 1048576
S = 32768
PTS_PER_CLUSTER = 32
FEATURE = 64
FG0 = 64
FG1 = 128
NCORES = 8
NPC = N // NCORES          # 131072 points per core
SPC = S // NCORES          # 4096 clusters per core
G = NPC // 1024            # 128 column-groups of 512
NCHUNK = 32                # psum chunks per core (each = 4 groups)

USE_F32R = True

_PROGRAM = None  # (nc, input_names) cache


def _build_program():
    from concourse import bacc, bass, tile

    mybir = bass.mybir
    f32 = mybir.dt.float32
    # float32r: fp32 bits, full-rate (1 cycle/row) PE mode. The BIR verifier
    # requires every producer of an f32r matmul operand to emit f32r, so the
    # DRAM tensors / SBUF tiles on matmul paths are declared f32r outright.
    fmm = mybir.dt.float32r if USE_F32R else f32
    AX = mybir.AxisListType

    nc = bacc.Bacc("TRN2", target_bir_lowering=False, debug=False)

    xt = nc.dram_tensor("xt", [6, G * 512], fmm, kind="ExternalInput").ap()
    wpack = nc.dram_tensor("wpack", [6, 128], fmm, kind="ExternalInput").ap()
    b1d = nc.dram_tensor("b1d", [128, 1], f32, kind="ExternalInput").ap()
    w1gbd = nc.dram_tensor("w1gbd", [128, 128], fmm, kind="ExternalInput").ap()
    b1gd = nc.dram_tensor("b1gd", [128, 1], f32, kind="ExternalInput").ap()
    w2gt = nc.dram_tensor("w2gt", [128, 128], fmm, kind="ExternalInput").ap()
    b2g = nc.dram_tensor("b2g", [128, 1], f32, kind="ExternalInput").ap()
    outA = nc.dram_tensor("outA", [128, 2048], f32, kind="ExternalOutput").ap()
    outB = nc.dram_tensor("outB", [128, 2048], f32, kind="ExternalOutput").ap()

    with tile.TileContext(nc) as tc:
        with (
            tc.tile_pool(name="w", bufs=1) as wp,
            tc.tile_pool(name="x", bufs=3) as xp,
            tc.tile_pool(name="acc", bufs=1) as accp,
            tc.tile_pool(name="ps", bufs=2, space=bass.MemorySpace.PSUM) as pp,
        ):
            wpack_t = wp.tile([6, 128], fmm, tag="wpack")
            b1d_t = wp.tile([128, 1], f32, tag="b1d")
            w1gbd_t = wp.tile([128, 128], fmm, tag="w1gbd")
            b1gd_t = wp.tile([128, 1], f32, tag="b1gd")
            w2gt_t = wp.tile([128, 128], fmm, tag="w2gt")
            b2g_t = wp.tile([128, 1], f32, tag="b2g")
            # weight DMAs on the Scalar queue (HWDGE); wpack first — it
            # gates the first matmul.
            for t, d in (
                (wpack_t, wpack),
                (b1d_t, b1d),
                (w1gbd_t, w1gbd),
                (b1gd_t, b1gd),
                (w2gt_t, w2gt),
                (b2g_t, b2g),
            ):
                nc.scalar.dma_start(t[:], d[:])

            pooled = accp.tile([128, NCHUNK, 4, 16], f32, tag="pooled")
            pooledR = accp.tile([128, 2048], fmm, tag="pooledR")

            # main loop: fc1 matmuls + segment-max pooling
            for k in range(8):  # 8 DMA chunks of [6, 8192]
                xt_t = xp.tile([6, 8192], fmm, tag="xt")
                if k == 0:
                    # split so the first matmul's columns land early
                    nc.sync.dma_start(xt_t[:, 0:2048], xt[:, 0:2048])
                    nc.sync.dma_start(xt_t[:, 2048:8192], xt[:, 2048:8192])
                else:
                    nc.sync.dma_start(xt_t[:], xt[:, k * 8192 : (k + 1) * 8192])
                for m in range(4):
                    i = 4 * k + m
                    ps = pp.tile([128, 4, 16, 32], f32, tag="ps")
                    for b in range(4):
                        c0 = (4 * m + b) * 512
                        nc.tensor.matmul(
                            ps[:, b],
                            wpack_t[:],
                            xt_t[:, c0 : c0 + 512],
                        )
                    # all reduces on DVE: it is the only engine with a
                    # free-axis max on this target (Pool/GPSIMD rejects
                    # TensorTensor/TensorReduce/InstPool at the ISA level)
                    nc.vector.reduce_max(pooled[:, i], ps[:], axis=AX.X)
                    if i % 8 == 2 and i > 8:
                        # relu(+b1) an eighth of pooled once its chunks are
                        # done; deferred two chunks so the ACT-queue wait
                        # can't stall the next eviction.
                        s = i // 8 - 1
                        nc.scalar.activation(
                            pooledR[:, s * 512 : (s + 1) * 512],
                            pooled[:, s * 8 : (s + 1) * 8],
                            mybir.ActivationFunctionType.Relu,
                            bias=b1d_t[:],
                        )

            # last eighth of pooledR
            nc.scalar.activation(
                pooledR[:, 1536:2048],
                pooled[:, 24:32],
                mybir.ActivationFunctionType.Relu,
                bias=b1d_t[:],
            )

            # tail MLP, pipelined in 512-col sub-slices
            hps = pp.tile([128, 4, 16, 32], f32, tag="ps")
            hR = accp.tile([128, 2048], fmm, tag="hR")
            for j in range(4):
                nc.tensor.matmul(
                    hps[:, j],
                    w1gbd_t[:],
                    pooledR[:, j * 512 : (j + 1) * 512],
                )
                nc.scalar.activation(
                    hR[:, j * 512 : (j + 1) * 512],
                    hps[:, j],
                    mybir.ActivationFunctionType.Relu,
                    bias=b1gd_t[:],
                )

            opsA = pp.tile([128, 4, 16, 32], f32, tag="ps")
            opsB = pp.tile([128, 4, 16, 32], f32, tag="ps")
            o2A = accp.tile([128, 2048], f32, tag="o2A")
            o2B = accp.tile([128, 2048], f32, tag="o2B")
            add = mybir.AluOpType.add
            vmax = mybir.AluOpType.max
            for j in range(4):
                nc.tensor.matmul(
                    opsA[:, j],
                    w2gt_t[0:64, :],
                    hR[0:64, j * 512 : (j + 1) * 512],
                )
                nc.tensor.matmul(
                    opsB[:, j],
                    w2gt_t[64:128, :],
                    hR[64:128, j * 512 : (j + 1) * 512],
                )
                # relu(+b2g): o2A + first half of o2B on DVE, rest on ACT
                nc.vector.tensor_scalar(
                    o2A[:, j * 512 : (j + 1) * 512],
                    opsA[:, j], b2g_t[:], 0.0, op0=add, op1=vmax,
                )
                if j < 2:
                    nc.vector.tensor_scalar(
                        o2B[:, j * 512 : (j + 1) * 512],
                        opsB[:, j], b2g_t[:], 0.0, op0=add, op1=vmax,
                    )
                else:
                    nc.scalar.activation(
                        o2B[:, j * 512 : (j + 1) * 512],
                        opsB[:, j],
                        mybir.ActivationFunctionType.Relu,
                        bias=b2g_t[:],
                    )
                if j == 1:
                    nc.sync.dma_start(outA[:, 0:1024], o2A[:, 0:1024])
                if j == 2:
                    # after the j==2 ACT so the issue's wait on DVE's
                    # o2B slices can't stall ACT compute
                    nc.scalar.dma_start(outB[:, 0:1024], o2B[:, 0:1024])
            nc.sync.dma_start(outA[:, 1024:2048], o2A[:, 1024:2048])
            nc.scalar.dma_start(outB[:, 1024:2048], o2B[:, 1024:2048])

    nc.compile()
    return nc


def _get_program():
    global _PROGRAM
    if _PROGRAM is None:
        _PROGRAM = _build_program()
    return _PROGRAM


def _host_pack(relative_points, W1, b1, W1g, b1g, W2g, b2g):
    X = np.ascontiguousarray(relative_points, dtype=np.float32)
    W1 = np.asarray(W1, np.float32)
    b1 = np.asarray(b1, np.float32)
    W1g = np.asarray(W1g, np.float32)
    b1g = np.asarray(b1g, np.float32)
    W2g = np.asarray(W2g, np.float32)
    b2g = np.asarray(b2g, np.float32)

    wpack = np.zeros((6, 128), np.float32)
    wpack[0:3, 0:64] = W1.T
    wpack[3:6, 64:128] = W1.T
    b1d = np.concatenate([b1, b1]).reshape(128, 1)
    w1gbd = np.zeros((128, 128), np.float32)
    w1gbd[0:64, 0:64] = W1g.T
    w1gbd[64:128, 64:128] = W1g.T
    b1gd = np.concatenate([b1g, b1g]).reshape(128, 1)
    w2gt = np.ascontiguousarray(np.vstack([W2g.T, W2g.T]))  # [128, 128]
    b2gc = np.ascontiguousarray(b2g.reshape(128, 1))

    in_maps = []
    for d in range(NCORES):
        Xc = X[d * NPC : (d + 1) * NPC]
        xt6 = np.ascontiguousarray(
            Xc.reshape(G, 2, 512, 3).transpose(1, 3, 0, 2).reshape(6, G * 512)
        )
        in_maps.append(
            {
                "xt": xt6,
                "wpack": wpack,
                "b1d": b1d,
                "w1gbd": w1gbd,
                "b1gd": b1gd,
                "w2gt": w2gt,
                "b2g": b2gc,
            }
        )
    return in_maps


def _host_unpack(results):
    out = np.empty((S, FG1), np.float32)
    for d in range(NCORES):
        oA = results[d]["outA"].reshape(128, NCHUNK, 4, 16)
        oB = results[d]["outB"].reshape(128, NCHUNK, 4, 16)
        blk = out[d * SPC : (d + 1) * SPC].reshape(NCHUNK, 4, 2, 16, 128)
        blk[:, :, 0] = oA.transpose(1, 2, 3, 0)
        blk[:, :, 1] = oB.transpose(1, 2, 3, 0)
    return out


def _numpy_fallback(relative_points, cluster, num_clusters,
                    W1, b1, W1g, b1g, W2g, b2g):
    X = np.asarray(relative_points, np.float32)
    fc1 = np.maximum(X @ np.asarray(W1, np.float32).T + np.asarray(b1, np.float32), 0.0)
    Sn = int(num_clusters)
    cl = np.asarray(cluster).astype(np.int64)
    pooled = np.full((Sn, fc1.shape[1]), -np.inf, np.float32)
    # sorted segment ids -> reduceat over run starts
    starts = np.flatnonzero(np.r_[True, cl[1:] != cl[:-1]])
    seg_ids = cl[starts]
    pooled[seg_ids] = np.maximum.reduceat(fc1, starts, axis=0)
    h = np.maximum(pooled @ np.asarray(W1g, np.float32).T + np.asarray(b1g, np.float32), 0.0)
    return np.maximum(h @ np.asarray(W2g, np.float32).T + np.asarray(b2g, np.float32), 0.0).astype(np.float32)


def _run_hw(in_maps, trace=False):
    from concourse.bass_utils import run_bass_kernel_spmd

    nc = _get_program()
    return run_bass_kernel_spmd(
        nc, in_maps, list(range(NCORES)), trace=trace
    )


def kernel(relative_points, cluster, num_clusters,
           W1, b1, W1g, b1g, W2g, b2g):
    cl = np.asarray(cluster)
    expected_cl = np.arange(N, dtype=np.int64) // PTS_PER_CLUSTER
    if (
        relative_points.shape != (N, 3)
        or int(num_clusters) != S
        or not np.array_equal(cl, expected_cl)
    ):
        return _numpy_fallback(relative_points, cluster, num_clusters,
                               W1, b1, W1g, b1g, W2g, b2g)

    in_maps = _host_pack(relative_points, W1, b1, W1g, b1g, W2g, b2g)
    res = _run_hw(in_maps, trace=False)
    return _host_unpack(res.results)


def run_traced(inputs):
    """test.py helper: returns (output, exec_time_ns)."""
    in_maps = _host_pack(
        inputs["relative_points"], inputs["W1"], inputs["b1"],
        inputs["W1g"], inputs["b1g"], inputs["W2g"], inputs["b2g"],
    )
    res = _run_hw(in_maps, trace=True)
    return _host_unpack(res.results), res.exec_time_ns



# revision 5
# speedup vs baseline: 1.1665x; 1.1665x over previous
"""Trainium2 Bass kernel for OldNeighborhoodEncoder (segment_reduce).

Math (reference):
    fc1    = relu(X @ W1.T + b1)            # [N, 64], X = [N, 3]
    pooled = segment_max(fc1, cluster, S)   # [S, 64], cluster = arange(N)//32
    h      = relu(pooled @ W1g.T + b1g)     # [S, 64]
    out    = relu(h @ W2g.T + b2g)          # [S, 128]

Hardcoded sizes: N=1048576, S=32768 (32 pts/cluster), FEATURE=64, FG0=64,
FG1=128, 8 cores. Data-parallel over points: core d handles points
[d*131072, (d+1)*131072) == clusters [d*4096, (d+1)*4096); no collectives.

v2 design (vs v1's single-engine DVE reduce_max at 1 elem/cycle/lane):
  * bias b1 is folded into the fc1 matmul as an extra all-ones moving row
    (K=8: xyz+1 for two 512-point sets), so pooling max runs on pre-relu
    values and relu is applied once at the end (relu o max == max o relu).
  * matmul moving data xt is bf16 [16, 32768] in DRAM: two 8-row blocks at
    SBUF partition bases {0, 32} (base must be 0/32/64), matmul m=2w+j
    takes moving xt_t[32j:32j+8, 512w:512(w+1)].  16 DMA lanes vs v1's 6,
    and the 2.6x fewer input bytes make the input stream a non-issue.
  * pooling is split across BOTH ACT and DVE ("routes"): 26 of 32 chunks
    (A-route) are drained PSUM->SBUF fp16 by ACT (activation Relu, 2048
    elem/lane @1.2GHz); the rest (B-route, k%5==3) are drained by DVE with
    one reduce_max over the [128,4,16,4,8] view straight into the staged
    L3 slot (walrus IBVF027 forbids two PSUM inputs, so no pair-max drain).
    All tree levels are fp16 tensor_tensor on DVE, which gets the 2x_1p
    DVE perf mode (2 elem/cycle/lane).
  * tree: L1 32->16, L2 16->8, L3 8->4 into a staged buffer
    [128, 8chunks, 4, 16, 4]; every 8 chunks two batched finals: F1 4->2
    (fp16 2x) and F2 2->1 fused with relu via scalar_tensor_tensor
    max(max(a,0),b) directly into pooled16.
  * tail MLP runs on fp16 pooled/h (PE rate for fp16 == f32r, fewer
    bytes), psum stays f32, biases f32; relu work split ACT/DVE as in v1.
"""

import sys
import numpy as np

if "/opt/trn_rl_repo" not in sys.path:
    sys.path.insert(0, "/opt/trn_rl_repo")

N = 1048576
S = 32768
PTS_PER_CLUSTER = 32
FEATURE = 64
FG0 = 64
FG1 = 128
NCORES = 8
NPC = N // NCORES          # 131072 points per core
SPC = S // NCORES          # 4096 clusters per core
NCHUNK = 32                # psum chunks per core (each = 4 matmuls of 512)

_PROGRAM = None  # (nc, input_names) cache


def _build_program():
    from concourse import bacc, bass, tile

    mybir = bass.mybir
    f32 = mybir.dt.float32
    bf16 = mybir.dt.bfloat16
    fp16 = mybir.dt.float16
    vmax = mybir.AluOpType.max
    vadd = mybir.AluOpType.add
    Relu = mybir.ActivationFunctionType.Relu
    AX = mybir.AxisListType

    nc = bacc.Bacc("TRN2", target_bir_lowering=False, debug=False)

    xtD = nc.dram_tensor("xtD", [16, 32768], bf16, kind="ExternalInput").ap()
    wrep = nc.dram_tensor("wrep", [128, 128], bf16, kind="ExternalInput").ap()
    w1gbd = nc.dram_tensor("w1gbd", [128, 128], fp16, kind="ExternalInput").ap()
    b1gd = nc.dram_tensor("b1gd", [128, 1], f32, kind="ExternalInput").ap()
    w2gt = nc.dram_tensor("w2gt", [128, 128], fp16, kind="ExternalInput").ap()
    b2g = nc.dram_tensor("b2g", [128, 1], f32, kind="ExternalInput").ap()
    outA = nc.dram_tensor("outA", [128, 2048], f32, kind="ExternalOutput").ap()
    outB = nc.dram_tensor("outB", [128, 2048], f32, kind="ExternalOutput").ap()

    with tile.TileContext(nc) as tc:
        with (
            tc.tile_pool(name="w", bufs=1) as wp,
            tc.tile_pool(name="x", bufs=1) as xp,
            tc.tile_pool(name="cv", bufs=3) as cvp,
            tc.tile_pool(name="t1", bufs=2) as t1p,
            tc.tile_pool(name="t2", bufs=2) as t2p,
            tc.tile_pool(name="st", bufs=2) as stp,
            tc.tile_pool(name="fin", bufs=2) as fip,
            tc.tile_pool(name="acc", bufs=1) as accp,
            tc.tile_pool(name="ps", bufs=2, space=bass.MemorySpace.PSUM) as pp,
        ):
            wrep_t = wp.tile([128, 128], bf16, tag="wrep")
            w1gbd_t = wp.tile([128, 128], fp16, tag="w1gbd")
            b1gd_t = wp.tile([128, 1], f32, tag="b1gd")
            w2gt_t = wp.tile([128, 128], fp16, tag="w2gt")
            b2g_t = wp.tile([128, 1], f32, tag="b2g")
            # weight DMAs on the Scalar queue (HWDGE); wrep first — it
            # gates the first matmul.
            for t, d in (
                (wrep_t, wrep),
                (w1gbd_t, w1gbd),
                (b1gd_t, b1gd),
                (w2gt_t, w2gt),
                (b2g_t, b2g),
            ):
                nc.scalar.dma_start(t[:], d[:])

            # input stream: one [128, 32768] bf16 tile, rows 0:8 and 32:40
            # used.  Pieces sized in chunks (1024 cols per chunk), small
            # first so the opening matmul isn't gated on a bulk transfer.
            xt_t = xp.tile([128, 32768], bf16, tag="xt")
            for c0, nch in ((0, 1), (1, 1), (2, 3), (5, 4), (9, 8), (17, 8), (25, 7)):
                cs = slice(1024 * c0, 1024 * (c0 + nch))
                nc.sync.dma_start(xt_t[0:8, cs], xtD[0:8, cs])
                nc.sync.dma_start(xt_t[32:40, cs], xtD[8:16, cs])

            # pooled16[p, v, kk, b, q]: pooled fc1 (post-relu) for cluster
            # 128*(8v+kk) + 32b + 16a + q, feature f, where p = 64a + f.
            pooled16 = accp.tile([128, 4, 8, 4, 16], fp16, tag="pooled16")

            stg_t = None
            for k in range(NCHUNK):
                ps = pp.tile([128, 4, 16, 32], f32, tag="ps")
                for b in range(4):
                    m = 4 * k + b
                    j, w = m % 2, m // 2
                    nc.tensor.matmul(
                        ps[:, b],
                        wrep_t[32 * j : 32 * j + 8, :],
                        xt_t[32 * j : 32 * j + 8, 512 * w : 512 * (w + 1)],
                    )
                if k % 8 == 0:
                    stg_t = stp.tile([128, 8, 4, 16, 4], fp16, tag="stg")
                if k % 5 != 3:
                    # A-route: ACT drains psum -> fp16 with relu fused
                    # (relu before max is fine: max is monotone, and the
                    # final relu at F2 is idempotent on these), then DVE
                    # runs the fp16 2x max tree L1-L3.
                    s16 = cvp.tile([128, 4, 16, 32], fp16, tag="s16")
                    nc.scalar.activation(s16[:], ps[:], Relu)
                    t1 = t1p.tile([128, 4, 16, 16], fp16, tag="t1")
                    nc.vector.tensor_tensor(
                        t1[:], s16[:, :, :, 0:16], s16[:, :, :, 16:32], vmax
                    )
                    t2 = t2p.tile([128, 4, 16, 8], fp16, tag="t2")
                    nc.vector.tensor_tensor(
                        t2[:], t1[:, :, :, 0:8], t1[:, :, :, 8:16], vmax
                    )
                    nc.vector.tensor_tensor(
                        stg_t[:, k % 8], t2[:, :, :, 0:4], t2[:, :, :, 4:8], vmax
                    )
                else:
                    # B-route: DVE drains psum with one 8-wide reduce_max
                    # (pre-relu values; F2's fused relu fixes them up).
                    nc.vector.reduce_max(
                        stg_t[:, k % 8],
                        ps[:, :, :, 0:32].rearrange("p a b (c d) -> p a b c d", c=4),
                        axis=AX.X,
                    )
                if k % 8 == 7:
                    fin = fip.tile([128, 8, 4, 16, 2], fp16, tag="fin")
                    nc.vector.tensor_tensor(
                        fin[:], stg_t[:, :, :, :, 0:2], stg_t[:, :, :, :, 2:4], vmax
                    )
                    # F2 + relu: max(max(a, 0), b)
                    nc.vector.scalar_tensor_tensor(
                        pooled16[:, k // 8],
                        fin[:, :, :, :, 0],
                        0.0,
                        fin[:, :, :, :, 1],
                        op0=vmax,
                        op1=vmax,
                    )

            # tail MLP, pipelined in 512-col sub-slices
            hps = pp.tile([128, 4, 16, 32], f32, tag="ps")
            hR = accp.tile([128, 2048], fp16, tag="hR")
            for j in range(4):
                nc.tensor.matmul(
                    hps[:, j],
                    w1gbd_t[:],
                    pooled16[:, j].rearrange("p a b c -> p (a b c)"),
                )
                nc.scalar.activation(
                    hR[:, j * 512 : (j + 1) * 512],
                    hps[:, j],
                    Relu,
                    bias=b1gd_t[:],
                )

            opsA = pp.tile([128, 4, 16, 32], f32, tag="ps")
            opsB = pp.tile([128, 4, 16, 32], f32, tag="ps")
            o2A = accp.tile([128, 2048], f32, tag="o2A")
            o2B = accp.tile([128, 2048], f32, tag="o2B")
            for j in range(4):
                nc.tensor.matmul(
                    opsA[:, j],
                    w2gt_t[0:64, :],
                    hR[0:64, j * 512 : (j + 1) * 512],
                )
                nc.tensor.matmul(
                    opsB[:, j],
                    w2gt_t[64:128, :],
                    hR[64:128, j * 512 : (j + 1) * 512],
                )
                # relu(+b2g): o2A + first half of o2B on DVE, rest on ACT
                nc.vector.tensor_scalar(
                    o2A[:, j * 512 : (j + 1) * 512],
                    opsA[:, j], b2g_t[:], 0.0, op0=vadd, op1=vmax,
                )
                if j < 2:
                    nc.vector.tensor_scalar(
                        o2B[:, j * 512 : (j + 1) * 512],
                        opsB[:, j], b2g_t[:], 0.0, op0=vadd, op1=vmax,
                    )
                else:
                    nc.scalar.activation(
                        o2B[:, j * 512 : (j + 1) * 512],
                        opsB[:, j],
                        Relu,
                        bias=b2g_t[:],
                    )
                if j == 1:
                    nc.sync.dma_start(outA[:, 0:1024], o2A[:, 0:1024])
                if j == 2:
                    nc.scalar.dma_start(outB[:, 0:1024], o2B[:, 0:1024])
            nc.sync.dma_start(outA[:, 1024:2048], o2A[:, 1024:2048])
            nc.scalar.dma_start(outB[:, 1024:2048], o2B[:, 1024:2048])

    nc.compile()
    return nc


def _get_program():
    global _PROGRAM
    if _PROGRAM is None:
        _PROGRAM = _build_program()
    return _PROGRAM


def _host_pack(relative_points, W1, b1, W1g, b1g, W2g, b2g):
    import ml_dtypes

    bf16 = ml_dtypes.bfloat16
    X = np.ascontiguousarray(relative_points, dtype=np.float32)
    W1 = np.asarray(W1, np.float32)
    b1 = np.asarray(b1, np.float32)
    W1g = np.asarray(W1g, np.float32)
    b1g = np.asarray(b1g, np.float32)
    W2g = np.asarray(W2g, np.float32)
    b2g = np.asarray(b2g, np.float32)

    # stationary block: rows 0-2 W1.T -> outs 0:64, row 3 b1; rows 4-7 the
    # same for outs 64:128.  Replicated at partition bases 0 and 32.
    blk = np.zeros((8, 128), np.float32)
    blk[0:3, 0:64] = W1.T
    blk[3, 0:64] = b1
    blk[4:7, 64:128] = W1.T
    blk[7, 64:128] = b1
    wrep = np.zeros((128, 128), np.float32)
    wrep[0:8] = blk
    wrep[32:40] = blk
    wrep = wrep.astype(bf16)

    w1gbd = np.zeros((128, 128), np.float32)
    w1gbd[0:64, 0:64] = W1g.T
    w1gbd[64:128, 64:128] = W1g.T
    w1gbd = w1gbd.astype(np.float16)
    b1gd = np.concatenate([b1g, b1g]).reshape(128, 1)
    w2gt = np.vstack([W2g.T, W2g.T]).astype(np.float16)  # [128, 128]
    b2gc = np.ascontiguousarray(b2g.reshape(128, 1))

    in_maps = []
    for d in range(NCORES):
        Xc = X[d * NPC : (d + 1) * NPC]
        # xt8[4h+r, m, o]: r=0..2 xyz of point 1024m+512h+o, r=3 ones
        t = Xc.reshape(128, 2, 512, 3).transpose(1, 3, 0, 2)  # [h,xyz,m,o]
        xt8 = np.empty((2, 4, 128, 512), np.float32)
        xt8[:, 0:3] = t
        xt8[:, 3] = 1.0
        # xtD[8j+r, 512w+o] = xt8[r, m=2w+j, o]
        xtD = np.ascontiguousarray(
            xt8.reshape(8, 64, 2, 512).transpose(2, 0, 1, 3).reshape(16, 32768)
        ).astype(bf16)
        in_maps.append(
            {
                "xtD": xtD,
                "wrep": wrep,
                "w1gbd": w1gbd,
                "b1gd": b1gd,
                "w2gt": w2gt,
                "b2g": b2gc,
            }
        )
    return in_maps


def _host_unpack(results):
    out = np.empty((S, FG1), np.float32)
    for d in range(NCORES):
        oA = results[d]["outA"].reshape(128, NCHUNK, 4, 16)
        oB = results[d]["outB"].reshape(128, NCHUNK, 4, 16)
        blk = out[d * SPC : (d + 1) * SPC].reshape(NCHUNK, 4, 2, 16, 128)
        blk[:, :, 0] = oA.transpose(1, 2, 3, 0)
        blk[:, :, 1] = oB.transpose(1, 2, 3, 0)
    return out


def _numpy_fallback(relative_points, cluster, num_clusters,
                    W1, b1, W1g, b1g, W2g, b2g):
    X = np.asarray(relative_points, np.float32)
    fc1 = np.maximum(X @ np.asarray(W1, np.float32).T + np.asarray(b1, np.float32), 0.0)
    Sn = int(num_clusters)
    cl = np.asarray(cluster).astype(np.int64)
    pooled = np.full((Sn, fc1.shape[1]), -np.inf, np.float32)
    # sorted segment ids -> reduceat over run starts
    starts = np.flatnonzero(np.r_[True, cl[1:] != cl[:-1]])
    seg_ids = cl[starts]
    pooled[seg_ids] = np.maximum.reduceat(fc1, starts, axis=0)
    h = np.maximum(pooled @ np.asarray(W1g, np.float32).T + np.asarray(b1g, np.float32), 0.0)
    return np.maximum(h @ np.asarray(W2g, np.float32).T + np.asarray(b2g, np.float32), 0.0).astype(np.float32)


def _run_hw(in_maps, trace=False):
    from concourse.bass_utils import run_bass_kernel_spmd

    nc = _get_program()
    return run_bass_kernel_spmd(
        nc, in_maps, list(range(NCORES)), trace=trace
    )


def kernel(relative_points, cluster, num_clusters,
           W1, b1, W1g, b1g, W2g, b2g):
    cl = np.asarray(cluster)
    expected_cl = np.arange(N, dtype=np.int64) // PTS_PER_CLUSTER
    if (
        relative_points.shape != (N, 3)
        or int(num_clusters) != S
        or not np.array_equal(cl, expected_cl)
    ):
        return _numpy_fallback(relative_points, cluster, num_clusters,
                               W1, b1, W1g, b1g, W2g, b2g)

    in_maps = _host_pack(relative_points, W1, b1, W1g, b1g, W2g, b2g)
    res = _run_hw(in_maps, trace=False)
    return _host_unpack(res.results)


def run_traced(inputs):
    """test.py helper: returns (output, exec_time_ns)."""
    in_maps = _host_pack(
        inputs["relative_points"], inputs["W1"], inputs["b1"],
        inputs["W1g"], inputs["b1g"], inputs["W2g"], inputs["b2g"],
    )
    res = _run_hw(in_maps, trace=True)
    return _host_unpack(res.results), res.exec_time_ns


# revision 9
# speedup vs baseline: 1.3445x; 1.1526x over previous
"""Trainium2 Bass kernel for OldNeighborhoodEncoder (segment_reduce).

Math (reference):
    fc1    = relu(X @ W1.T + b1)            # [N, 64], X = [N, 3]
    pooled = segment_max(fc1, cluster, S)   # [S, 64], cluster = arange(N)//32
    h      = relu(pooled @ W1g.T + b1g)     # [S, 64]
    out    = relu(h @ W2g.T + b2g)          # [S, 128]

Hardcoded sizes: N=1048576, S=32768 (32 pts/cluster), FEATURE=64, FG0=64,
FG1=128, 8 cores. Data-parallel over points: core d handles points
[d*131072, (d+1)*131072) == clusters [d*4096, (d+1)*4096); no collectives.

v2 design (vs v1's single-engine DVE reduce_max at 1 elem/cycle/lane):
  * bias b1 is folded into the fc1 matmul as an extra all-ones moving row
    (K=8: xyz+1 for two 512-point sets), so pooling max runs on pre-relu
    values and relu is applied once at the end (relu o max == max o relu).
  * matmul moving data xt is bf16 [16, 32768] in DRAM: two 8-row blocks at
    SBUF partition bases {0, 32} (base must be 0/32/64), matmul m=2w+j
    takes moving xt_t[32j:32j+8, 512w:512(w+1)].  16 DMA lanes vs v1's 6,
    and the 2.6x fewer input bytes make the input stream a non-issue.
  * PSUM can only be read by ACT and DVE (walrus: GPSIMD cannot access
    PSUM, DMA source must be SBUF/DRAM, and no instruction may read two
    PSUM operands), so pooling is split across exactly those two: 6 of
    every 8 chunks (A-route) are drained PSUM->SBUF fp16 by ACT
    (activation Relu, ~2.29us measured); the last 2 of each group
    (B-route) are drained by DVE with one reduce_max over the
    [128,4,16,4,8] view straight into the staged L3 slot (~2.75us), and
    are placed at group positions 6,7 so their psum frees immediately
    (the batched tree would otherwise sit in front of them in the DVE
    queue and stall the PE).
  * A-route tree is batched per group of 8 chunks to amortize the ~190ns
    DVE instruction overhead: drains land in s8[128,6,4,16,32], then one
    L1 (6144 charged elems, fp16 2x), one L2, one L3 into
    staged[128,8,4,16,4]; F1 4->2 (fp16 2x) and F2 2->1 fused with relu
    via scalar_tensor_tensor max(max(a,0),b) write pooled16.  The last
    group runs the tree per-chunk instead (3 small ops after each drain)
    so only ~3us of tree work trails the final drain.
  * tail MLP runs on fp16 pooled/h (PE rate for fp16 == f32r, fewer
    bytes), psum stays f32, biases f32; relu work split ACT/DVE as in v1.
"""

import sys
import numpy as np

if "/opt/trn_rl_repo" not in sys.path:
    sys.path.insert(0, "/opt/trn_rl_repo")

N = 1048576
S = 32768
PTS_PER_CLUSTER = 32
FEATURE = 64
FG0 = 64
FG1 = 128
NCORES = 8
NPC = N // NCORES          # 131072 points per core
SPC = S // NCORES          # 4096 clusters per core
NCHUNK = 32                # psum chunks per core (each = 4 matmuls of 512)

_PROGRAM = None  # (nc, input_names) cache


def _build_program():
    from concourse import bacc, bass, tile

    mybir = bass.mybir
    f32 = mybir.dt.float32
    bf16 = mybir.dt.bfloat16
    fp16 = mybir.dt.float16
    vmax = mybir.AluOpType.max
    vadd = mybir.AluOpType.add
    Relu = mybir.ActivationFunctionType.Relu
    AX = mybir.AxisListType

    nc = bacc.Bacc("TRN2", target_bir_lowering=False, debug=False)

    xtD = nc.dram_tensor("xtD", [16, 32768], bf16, kind="ExternalInput").ap()
    wrep = nc.dram_tensor("wrep", [128, 128], bf16, kind="ExternalInput").ap()
    w1gbd = nc.dram_tensor("w1gbd", [128, 128], fp16, kind="ExternalInput").ap()
    b1gd = nc.dram_tensor("b1gd", [128, 1], f32, kind="ExternalInput").ap()
    w2gt = nc.dram_tensor("w2gt", [128, 128], fp16, kind="ExternalInput").ap()
    b2g = nc.dram_tensor("b2g", [128, 1], f32, kind="ExternalInput").ap()
    outA = nc.dram_tensor("outA", [128, 2048], f32, kind="ExternalOutput").ap()
    outB = nc.dram_tensor("outB", [128, 2048], f32, kind="ExternalOutput").ap()

    with tile.TileContext(nc) as tc:
        with (
            tc.tile_pool(name="w", bufs=1) as wp,
            tc.tile_pool(name="x", bufs=1) as xp,
            tc.tile_pool(name="cv", bufs=2) as cvp,
            tc.tile_pool(name="t1", bufs=2) as t1p,
            tc.tile_pool(name="t2", bufs=2) as t2p,
            tc.tile_pool(name="st", bufs=2) as stp,
            tc.tile_pool(name="fin", bufs=2) as fip,
            tc.tile_pool(name="acc", bufs=1) as accp,
            tc.tile_pool(name="ps", bufs=2, space=bass.MemorySpace.PSUM) as pp,
        ):
            wrep_t = wp.tile([128, 128], bf16, tag="wrep")
            w1gbd_t = wp.tile([128, 128], fp16, tag="w1gbd")
            b1gd_t = wp.tile([128, 1], f32, tag="b1gd")
            w2gt_t = wp.tile([128, 128], fp16, tag="w2gt")
            b2g_t = wp.tile([128, 1], f32, tag="b2g")
            # weight DMAs on the Scalar queue (HWDGE); wrep first — it
            # gates the first matmul.
            for t, d in (
                (wrep_t, wrep),
                (w1gbd_t, w1gbd),
                (b1gd_t, b1gd),
                (w2gt_t, w2gt),
                (b2g_t, b2g),
            ):
                nc.scalar.dma_start(t[:], d[:])

            # input stream: one [128, 32768] bf16 tile, rows 0:8 and 32:40
            # used.  Pieces sized in half-chunks (512 cols), small first so
            # the opening matmul isn't gated on a bulk transfer.
            xt_t = xp.tile([128, 32768], bf16, tag="xt")
            for c0, nhc in ((0, 1), (1, 1), (2, 2), (4, 6), (10, 8), (18, 14),
                            (32, 16), (48, 16)):
                cs = slice(512 * c0, 512 * (c0 + nhc))
                nc.sync.dma_start(xt_t[0:8, cs], xtD[0:8, cs])
                nc.sync.dma_start(xt_t[32:40, cs], xtD[8:16, cs])

            # pooled16[p, v, kk, b, q]: pooled fc1 (post-relu) for cluster
            # 128*(8v+kk) + 32b + 16a + q, feature f, where p = 64a + f.
            pooled16 = accp.tile([128, 4, 8, 4, 16], fp16, tag="pooled16")

            for g in range(4):
                last = g == 3
                s8 = cvp.tile([128, 6, 4, 16, 32], fp16, tag="s8")
                stg_t = stp.tile([128, 8, 4, 16, 4], fp16, tag="stg")
                for i in range(8):
                    k = 8 * g + i
                    ps = pp.tile([128, 4, 16, 32], f32, tag="ps")
                    for b in range(4):
                        m = 4 * k + b
                        j, w = m % 2, m // 2
                        nc.tensor.matmul(
                            ps[:, b],
                            wrep_t[32 * j : 32 * j + 8, :],
                            xt_t[32 * j : 32 * j + 8, 512 * w : 512 * (w + 1)],
                        )
                    if i < 6:
                        # A-route: ACT drains psum -> fp16 with relu fused
                        # (relu before max is fine: max is monotone, and
                        # the final relu at F2 is idempotent on these).
                        nc.scalar.activation(s8[:, i], ps[:], Relu)
                        if last:
                            # per-chunk tree: minimizes work trailing the
                            # final drain
                            t1 = t1p.tile([128, 4, 16, 16], fp16, tag="t1")
                            nc.vector.tensor_tensor(
                                t1[:], s8[:, i, :, :, 0:16],
                                s8[:, i, :, :, 16:32], vmax
                            )
                            t2 = t2p.tile([128, 4, 16, 8], fp16, tag="t2")
                            nc.vector.tensor_tensor(
                                t2[:], t1[:, :, :, 0:8], t1[:, :, :, 8:16], vmax
                            )
                            nc.vector.tensor_tensor(
                                stg_t[:, i], t2[:, :, :, 0:4], t2[:, :, :, 4:8],
                                vmax
                            )
                    else:
                        # B-route: DVE drains psum with one 8-wide
                        # reduce_max (pre-relu values; F2's fused relu
                        # fixes them up).
                        nc.vector.reduce_max(
                            stg_t[:, i],
                            ps[:].rearrange("p a b (c d) -> p a b c d", c=4),
                            axis=AX.X,
                        )
                if not last:
                    # batched A-tree for slots 0..5, emitted after the
                    # B-reduces so their psum tiles free first
                    t1b = t1p.tile([128, 6, 4, 16, 16], fp16, tag="t1b")
                    nc.vector.tensor_tensor(
                        t1b[:], s8[:, :, :, :, 0:16], s8[:, :, :, :, 16:32], vmax
                    )
                    t2b = t2p.tile([128, 6, 4, 16, 8], fp16, tag="t2b")
                    nc.vector.tensor_tensor(
                        t2b[:], t1b[:, :, :, :, 0:8], t1b[:, :, :, :, 8:16], vmax
                    )
                    nc.vector.tensor_tensor(
                        stg_t[:, 0:6], t2b[:, :, :, :, 0:4], t2b[:, :, :, :, 4:8],
                        vmax
                    )
                fin = fip.tile([128, 8, 4, 16, 2], fp16, tag="fin")
                nc.vector.tensor_tensor(
                    fin[:], stg_t[:, :, :, :, 0:2], stg_t[:, :, :, :, 2:4], vmax
                )
                # F2 + relu: max(max(a, 0), b)
                nc.vector.scalar_tensor_tensor(
                    pooled16[:, g],
                    fin[:, :, :, :, 0],
                    0.0,
                    fin[:, :, :, :, 1],
                    op0=vmax,
                    op1=vmax,
                )

            # tail MLP, pipelined in 512-col sub-slices
            hps = pp.tile([128, 4, 16, 32], f32, tag="ps")
            hR = accp.tile([128, 2048], fp16, tag="hR")
            for j in range(4):
                nc.tensor.matmul(
                    hps[:, j],
                    w1gbd_t[:],
                    pooled16[:, j].rearrange("p a b c -> p (a b c)"),
                )
                nc.scalar.activation(
                    hR[:, j * 512 : (j + 1) * 512],
                    hps[:, j],
                    Relu,
                    bias=b1gd_t[:],
                )

            opsA = pp.tile([128, 4, 16, 32], f32, tag="ps")
            opsB = pp.tile([128, 4, 16, 32], f32, tag="ps")
            o2A = accp.tile([128, 2048], f32, tag="o2A")
            o2B = accp.tile([128, 2048], f32, tag="o2B")
            for j in range(4):
                nc.tensor.matmul(
                    opsA[:, j],
                    w2gt_t[0:64, :],
                    hR[0:64, j * 512 : (j + 1) * 512],
                )
                nc.tensor.matmul(
                    opsB[:, j],
                    w2gt_t[64:128, :],
                    hR[64:128, j * 512 : (j + 1) * 512],
                )
                # relu(+b2g): o2A + first half of o2B on DVE, rest on ACT
                nc.vector.tensor_scalar(
                    o2A[:, j * 512 : (j + 1) * 512],
                    opsA[:, j], b2g_t[:], 0.0, op0=vadd, op1=vmax,
                )
                if j < 2:
                    nc.vector.tensor_scalar(
                        o2B[:, j * 512 : (j + 1) * 512],
                        opsB[:, j], b2g_t[:], 0.0, op0=vadd, op1=vmax,
                    )
                else:
                    nc.scalar.activation(
                        o2B[:, j * 512 : (j + 1) * 512],
                        opsB[:, j],
                        Relu,
                        bias=b2g_t[:],
                    )
                if j == 1:
                    nc.sync.dma_start(outA[:, 0:1024], o2A[:, 0:1024])
                if j == 2:
                    nc.scalar.dma_start(outB[:, 0:1024], o2B[:, 0:1024])
            nc.sync.dma_start(outA[:, 1024:2048], o2A[:, 1024:2048])
            nc.scalar.dma_start(outB[:, 1024:2048], o2B[:, 1024:2048])

    nc.compile()
    return nc


def _get_program():
    global _PROGRAM
    if _PROGRAM is None:
        _PROGRAM = _build_program()
    return _PROGRAM


def _host_pack(relative_points, W1, b1, W1g, b1g, W2g, b2g):
    import ml_dtypes

    bf16 = ml_dtypes.bfloat16
    X = np.ascontiguousarray(relative_points, dtype=np.float32)
    W1 = np.asarray(W1, np.float32)
    b1 = np.asarray(b1, np.float32)
    W1g = np.asarray(W1g, np.float32)
    b1g = np.asarray(b1g, np.float32)
    W2g = np.asarray(W2g, np.float32)
    b2g = np.asarray(b2g, np.float32)

    # stationary block: rows 0-2 W1.T -> outs 0:64, row 3 b1; rows 4-7 the
    # same for outs 64:128.  Replicated at partition bases 0 and 32.
    blk = np.zeros((8, 128), np.float32)
    blk[0:3, 0:64] = W1.T
    blk[3, 0:64] = b1
    blk[4:7, 64:128] = W1.T
    blk[7, 64:128] = b1
    wrep = np.zeros((128, 128), np.float32)
    wrep[0:8] = blk
    wrep[32:40] = blk
    wrep = wrep.astype(bf16)

    w1gbd = np.zeros((128, 128), np.float32)
    w1gbd[0:64, 0:64] = W1g.T
    w1gbd[64:128, 64:128] = W1g.T
    w1gbd = w1gbd.astype(np.float16)
    b1gd = np.concatenate([b1g, b1g]).reshape(128, 1)
    w2gt = np.vstack([W2g.T, W2g.T]).astype(np.float16)  # [128, 128]
    b2gc = np.ascontiguousarray(b2g.reshape(128, 1))

    in_maps = []
    for d in range(NCORES):
        Xc = X[d * NPC : (d + 1) * NPC]
        # xt8[4h+r, m, o]: r=0..2 xyz of point 1024m+512h+o, r=3 ones
        t = Xc.reshape(128, 2, 512, 3).transpose(1, 3, 0, 2)  # [h,xyz,m,o]
        xt8 = np.empty((2, 4, 128, 512), np.float32)
        xt8[:, 0:3] = t
        xt8[:, 3] = 1.0
        # xtD[8j+r, 512w+o] = xt8[r, m=2w+j, o]
        xtD = np.ascontiguousarray(
            xt8.reshape(8, 64, 2, 512).transpose(2, 0, 1, 3).reshape(16, 32768)
        ).astype(bf16)
        in_maps.append(
            {
                "xtD": xtD,
                "wrep": wrep,
                "w1gbd": w1gbd,
                "b1gd": b1gd,
                "w2gt": w2gt,
                "b2g": b2gc,
            }
        )
    return in_maps


def _host_unpack(results):
    out = np.empty((S, FG1), np.float32)
    for d in range(NCORES):
        oA = results[d]["outA"].reshape(128, NCHUNK, 4, 16)
        oB = results[d]["outB"].reshape(128, NCHUNK, 4, 16)
        blk = out[d * SPC : (d + 1) * SPC].reshape(NCHUNK, 4, 2, 16, 128)
        blk[:, :, 0] = oA.transpose(1, 2, 3, 0)
        blk[:, :, 1] = oB.transpose(1, 2, 3, 0)
    return out


def _numpy_fallback(relative_points, cluster, num_clusters,
                    W1, b1, W1g, b1g, W2g, b2g):
    X = np.asarray(relative_points, np.float32)
    fc1 = np.maximum(X @ np.asarray(W1, np.float32).T + np.asarray(b1, np.float32), 0.0)
    Sn = int(num_clusters)
    cl = np.asarray(cluster).astype(np.int64)
    pooled = np.full((Sn, fc1.shape[1]), -np.inf, np.float32)
    # sorted segment ids -> reduceat over run starts
    starts = np.flatnonzero(np.r_[True, cl[1:] != cl[:-1]])
    seg_ids = cl[starts]
    pooled[seg_ids] = np.maximum.reduceat(fc1, starts, axis=0)
    h = np.maximum(pooled @ np.asarray(W1g, np.float32).T + np.asarray(b1g, np.float32), 0.0)
    return np.maximum(h @ np.asarray(W2g, np.float32).T + np.asarray(b2g, np.float32), 0.0).astype(np.float32)


def _run_hw(in_maps, trace=False):
    from concourse.bass_utils import run_bass_kernel_spmd

    nc = _get_program()
    return run_bass_kernel_spmd(
        nc, in_maps, list(range(NCORES)), trace=trace
    )


def kernel(relative_points, cluster, num_clusters,
           W1, b1, W1g, b1g, W2g, b2g):
    cl = np.asarray(cluster)
    expected_cl = np.arange(N, dtype=np.int64) // PTS_PER_CLUSTER
    if (
        relative_points.shape != (N, 3)
        or int(num_clusters) != S
        or not np.array_equal(cl, expected_cl)
    ):
        return _numpy_fallback(relative_points, cluster, num_clusters,
                               W1, b1, W1g, b1g, W2g, b2g)

    in_maps = _host_pack(relative_points, W1, b1, W1g, b1g, W2g, b2g)
    res = _run_hw(in_maps, trace=False)
    return _host_unpack(res.results)


def run_traced(inputs):
    """test.py helper: returns (output, exec_time_ns)."""
    in_maps = _host_pack(
        inputs["relative_points"], inputs["W1"], inputs["b1"],
        inputs["W1g"], inputs["b1g"], inputs["W2g"], inputs["b2g"],
    )
    res = _run_hw(in_maps, trace=True)
    return _host_unpack(res.results), res.exec_time_ns
